# revision 23
# baseline (speedup 1.0000x reference)
"""GCN (2-layer) + global mean pool on 8 Trainium2 NeuronCores.

Strategy
--------
Nodes are padded to 100352 = 784 tiles of 128; dest tiles are split
contiguously across 8 cores (98 each).  Per layer the replicated fp16 table

    g[n] = dinv[n] * h[n]        ([100352, 128], AllGather per layer)

is gathered per edge with the GPSIMD `dma_gather` custom DMA (int16 indices,
4 SWDGE queues in parallel).  int16 only reaches 65536 rows, so rows are
addressed in 512-byte pairs with a signed-window trick: the AP base sits at
pair 32768 and idx = node//2 - 32768 (parity picks the 256B half); edges are
split per dest tile into even/odd-source streams so every 2048-index call is
parity-pure.

Edges sorted by destination are processed in 128-edge chunks: the 0/1
selection matrix S[e, c] = (lid[e] == c) is built on the vector engine from a
tiled iota; aggregation is a PE matmul accumulated per dest-tile in PSUM.
deg^-1/2 is folded into the table (source side) and activation scales /
K=1 bias matmuls (dest side), so no per-edge multiply exists anywhere.

Layer 1 stores relu(dinv^2*agg + dinv*b1) as the next table; layer 2
aggregates transposed (lhsT=msg, rhs=S) so W2 consumes the aggregate as
stationary operand; mean pooling is one more indicator matmul accumulated
over all tiles in PSUM; the host sums the 8 per-core [64,128] partials.
"""

import sys

if '/opt/trn_rl_repo' not in sys.path:
    sys.path.insert(0, '/opt/trn_rl_repo')

import numpy as np

NCORES = 8
N = 100000
NP = 100352          # 784 tiles of 128
NTILES = 784
TPC = NTILES // NCORES   # 98 dest tiles per core
NPC = TPC * 128          # 12544 nodes per core
D = 512
H = 128
NG = 64
CPC = 8              # chunks per gather call / S-build batch (1024 idxs;
                     # 64 descs/engine = single-packet ceiling)
NSTR = 2             # gather streams (source parity)

_cache = {}


def _wrap_idx(vals16, tc_pad):
    """[128, tc_pad] int16 per-(lane, chunk) values -> dma_gather wrapped
    layout [128, tc_pad*8]: within-call index i = (q%16)*128 + p lives at
    [p%16 (+16r), q*8 + p//16]."""
    lanes = np.arange(128)
    out16 = np.zeros((16, tc_pad * 8), np.int16)
    cols = (lanes[:, None] // 16) + np.arange(tc_pad)[None, :] * 8
    rows = (lanes % 16)[:, None].repeat(tc_pad, axis=1)
    out16[rows, cols] = vals16
    return np.tile(out16, (8, 1))


def _host_prep_graph(edge_index, batch):
    ei = np.asarray(edge_index)
    row = np.concatenate([ei[0], np.arange(N, dtype=np.int64)])
    col = np.concatenate([ei[1], np.arange(N, dtype=np.int64)])
    deg = np.bincount(col, minlength=N).astype(np.float64)
    dinv = 1.0 / np.sqrt(deg)

    # ---- balanced dest-tile assignment (LPT), slot-paired by size ----
    tile_tot = np.bincount(col >> 7, minlength=NTILES)
    order_t = np.argsort(-tile_tot, kind='stable')
    loads = np.zeros(NCORES, np.int64)
    counts = np.zeros(NCORES, np.int64)
    assign = [[] for _ in range(NCORES)]
    for t in order_t:
        cand = np.where(counts < TPC)[0]
        cbest = cand[np.argmin(loads[cand])]
        assign[cbest].append(int(t))
        loads[cbest] += tile_tot[t]
        counts[cbest] += 1
    # big tiles first within each core -> slot i pairs similar sizes
    slot_tile = np.array([sorted(a, key=lambda t: -tile_tot[t])
                          for a in assign])          # [NCORES, TPC]

    # ---- node -> table-row remap (half-major, slot order per core) ----
    HT = TPC // 2
    NH = HT * 128
    HNP = NP // 2
    remap = np.zeros(NP, np.int64)
    for c in range(NCORES):
        for i in range(TPC):
            t = slot_tile[c, i]
            half = 1 if i >= HT else 0
            base = half * HNP + c * NH + (i - half * HT) * 128
            remap[t * 128:(t + 1) * 128] = base + np.arange(128)

    # ---- edge streams: (source table half)*2 + source parity ----
    rr = remap[row]
    es = (rr >= HNP) * 2 + (row & 1)
    tile_of_e = col >> 7
    key = tile_of_e * 4 + es
    order = np.argsort(key, kind='stable')
    row_s = row[order].astype(np.int64)
    col_s = col[order].astype(np.int64)
    es_s = es[order]

    tp = (col_s >> 7) * 4 + es_s
    tp_cnt = np.bincount(tp, minlength=NTILES * 4).reshape(NTILES, 4)
    tp_start = np.zeros(NTILES * 4 + 1, np.int64)
    np.cumsum(tp_cnt.reshape(-1), out=tp_start[1:])
    tp_start = tp_start[:-1].reshape(NTILES, 4)

    def pad16(x):
        return ((x + CPC - 1) // CPC) * CPC

    # chunks are laid out in the device's processing order: half-B slots
    # first, then half-A (matches TILE_ORDER in _build_nc)
    proc_order = list(range(HT, TPC)) + list(range(HT))
    cslot, tc_pad, bases = [], [], []
    for h in range(4):
        cnt_h = tp_cnt[slot_tile, h]                 # [NCORES, TPC]
        cs = np.ceil(cnt_h / 128).astype(np.int64).max(axis=0)
        cslot.append(cs)
        tc_pad.append(pad16(max(int(cs.sum()), 1)))
        b = np.zeros(TPC, np.int64)
        acc = 0
        for i in proc_order:
            b[i] = acc
            acc += cs[i]
        bases.append(b)

    idxs = [np.zeros((NCORES, 128, tc_pad[h]), np.int16) for h in range(4)]
    lids = [np.full((NCORES, 128, tc_pad[h]), -1.0, np.float16)
            for h in range(4)]
    for c in range(NCORES):
        for i in range(TPC):
            t = slot_tile[c, i]
            for h in range(4):
                s, n = tp_start[t, h], tp_cnt[t, h]
                if n == 0:
                    continue
                src = row_s[s:s + n]
                loc = (col_s[s:s + n] - (t << 7)).astype(np.float16)
                j0 = bases[h][i]
                ch = np.arange(n) // 128 + j0
                ln = np.arange(n) % 128
                idxs[h][c, ln, ch] = (remap[src] % HNP) >> 1
                lids[h][c, ln, ch] = loc

    # Padding slots (lid == -1) keep idx 0 by construction, which makes every
    # padding descriptor hammer table row 0 (one HBM bank).  Forward-fill them
    # with the preceding real index in unwrapped (chunk, lane) order so the
    # padding fetch hits the row the engine just read (open-row hit).
    for h in range(4):
        for c in range(NCORES):
            flat_i = idxs[h][c].T.reshape(-1).copy()      # (ch, ln) order
            flat_m = (lids[h][c].T.reshape(-1) == -1.0)
            last = np.where(~flat_m, np.arange(flat_i.size), -1)
            np.maximum.accumulate(last, out=last)
            src = np.where(last >= 0, flat_i[np.maximum(last, 0)], flat_i)
            flat_i[flat_m] = src[flat_m]
            idxs[h][c] = flat_i.reshape(tc_pad[h], 128).T

    idx_w = [np.stack([_wrap_idx(idxs[h][c], tc_pad[h])
                       for c in range(NCORES)]) for h in range(4)]

    # ---- per-node dest-side arrays, in slot order per core ----
    dinv_p = np.ones(NP, np.float64)
    dinv_p[:N] = dinv
    rdinv_n = np.zeros(NP, np.float16)
    rdinv_n[:N] = (1.0 / dinv).astype(np.float16)
    dinv2_n = (dinv_p ** 2).astype(np.float32)
    dinv1_n = dinv_p.astype(np.float32)
    b = np.asarray(batch).astype(np.int64)
    cnt_g = np.bincount(b, minlength=NG).astype(np.float64)
    invcnt = 1.0 / np.maximum(cnt_g, 1.0)
    pmat_n = np.zeros((NP, NG), np.float16)
    pmat_n[np.arange(N), b] = invcnt[b].astype(np.float16)

    # node order per core = concat of its slot tiles
    node_order = np.zeros((NCORES, NPC), np.int64)
    for c in range(NCORES):
        node_order[c] = (slot_tile[c][:, None] * 128 +
                         np.arange(128)[None, :]).reshape(-1)

    return dict(
        dinv=dinv, cslot=cslot, tc_pad=tc_pad, idx=idx_w, lid=lids,
        node_order=node_order,
        rdinv=rdinv_n, dinv2=dinv2_n, dinv1=dinv1_n, pmat=pmat_n,
    )


def _build_nc(gp):
    import concourse.bass as bass
    import concourse.bacc as bacc
    import concourse.mybir as mybir
    import concourse.tile as tile

    fp16 = mybir.dt.float16
    fp8 = mybir.dt.float8e4
    fp32 = mybir.dt.float32
    i16 = mybir.dt.int16
    Relu = mybir.ActivationFunctionType.Relu
    iseq = mybir.AluOpType.is_equal

    tc_pad = gp['tc_pad']
    cslot = gp['cslot']
    STREAM_ORDER = (2, 3, 0, 1)      # half-B streams first (its AG lands first)

    nc = bacc.Bacc("TRN2", target_bir_lowering=False, debug=False,
                   num_devices=NCORES, num_swdge_queues=4)

    xT_in = nc.dram_tensor("xT", [D, NPC], fp16, kind="ExternalInput").ap()
    idx_in = [nc.dram_tensor(f"idx{h}", [128, tc_pad[h] * 8], i16,
                             kind="ExternalInput").ap() for h in range(4)]
    lid_in = [nc.dram_tensor(f"lid{h}", [128, tc_pad[h]], fp16,
                             kind="ExternalInput").ap() for h in range(4)]
    ident_in = nc.dram_tensor("ident", [128, 128], fp16,
                              kind="ExternalInput").ap()
    w1_in = nc.dram_tensor("w1", [128, 4 * 128], fp16, kind="ExternalInput").ap()
    w2_in = nc.dram_tensor("w2", [128, 128], fp16, kind="ExternalInput").ap()
    b1_in = nc.dram_tensor("b1", [1, 128], fp16, kind="ExternalInput").ap()
    b2_in = nc.dram_tensor("b2", [1, 128], fp16, kind="ExternalInput").ap()
    rdinv_in = nc.dram_tensor("rdinv", [1, NPC], fp16, kind="ExternalInput").ap()
    dinv2_in = nc.dram_tensor("dinv2", [128, TPC], fp32, kind="ExternalInput").ap()
    dinv1_in = nc.dram_tensor("dinv1", [128, TPC], fp32, kind="ExternalInput").ap()
    pmat_in = nc.dram_tensor("pmat", [128, TPC * NG], fp16,
                             kind="ExternalInput").ap()
    iota_in = nc.dram_tensor("iota", [128, CPC * 128], fp16,
                             kind="ExternalInput").ap()
    out_dram = nc.dram_tensor("pooled", [64, 128], fp32,
                              kind="ExternalOutput").ap()

    with tile.TileContext(nc) as tcx:
        import contextlib
        ctx = contextlib.ExitStack()
        with ctx:
            dram = ctx.enter_context(tcx.tile_pool(name="dram", bufs=1, space="DRAM"))
            cpool = ctx.enter_context(tcx.tile_pool(name="const", bufs=1))
            xt_pool = ctx.enter_context(tcx.tile_pool(name="xt", bufs=4))
            g0sb_pool = ctx.enter_context(tcx.tile_pool(name="g0sb", bufs=3))
            msg_pool = ctx.enter_context(tcx.tile_pool(name="msg", bufs=26))
            s_pool = ctx.enter_context(tcx.tile_pool(name="spool", bufs=26))
            h1sb_pool = ctx.enter_context(tcx.tile_pool(name="h1sb", bufs=3))
            part_pool = ctx.enter_context(tcx.tile_pool(name="part", bufs=TPC))
            aggt_pool = ctx.enter_context(tcx.tile_pool(name="aggt", bufs=3))
            h2sb_pool = ctx.enter_context(tcx.tile_pool(name="h2sb", bufs=3))
            osb_pool = ctx.enter_context(tcx.tile_pool(name="osb", bufs=1))
            psA = ctx.enter_context(tcx.tile_pool(name="psA", bufs=5, space="PSUM"))
            psB = ctx.enter_context(tcx.tile_pool(name="psB", bufs=2, space="PSUM"))
            psP = ctx.enter_context(tcx.tile_pool(name="psP", bufs=1, space="PSUM"))

            HT = TPC // 2
            NH = HT * 128
            HNP = NP // 2
            g_loc = {}
            g_half = {}
            for L in (0, 1):
                for hf in (0, 1):
                    g_loc[(L, hf)] = dram.tile([NH, 128], fp16,
                                               name=f"g{L}loc{hf}")
                    g_half[(L, hf)] = dram.tile([HNP, 128], fp16,
                                                addr_space="Shared",
                                                name=f"g{L}half{hf}")

            def cload(name, ap_in, shape, dt):
                t = cpool.tile(shape, dt, name=name)
                nc.sync.dma_start(out=t[:], in_=ap_in)
                return t

            ident_sb = cload("ident_sb", ident_in, [128, 128], fp16)
            idx_sb = [cload(f"idx_sb{h}", idx_in[h], [128, tc_pad[h] * 8], i16)
                      for h in range(4)]
            lid_sb = [cload(f"lid_sb{h}", lid_in[h], [128, tc_pad[h]], fp16)
                      for h in range(4)]
            w1_sb = cload("w1_sb", w1_in, [128, 4 * 128], fp16)
            w2_sb = cload("w2_sb", w2_in, [128, 128], fp16)
            b1_sb = cload("b1_sb", b1_in, [1, 128], fp16)
            b2_sb = cload("b2_sb", b2_in, [1, 128], fp16)
            rdinv_sb = cload("rdinv_sb", rdinv_in, [1, NPC], fp16)
            dinv2_sb = cload("dinv2_sb", dinv2_in, [128, TPC], fp32)
            dinv1_sb = cload("dinv1_sb", dinv1_in, [128, TPC], fp32)
            pmat_sb = cload("pmat_sb", pmat_in, [128, TPC * NG], fp16)
            iota_sb = cload("iota_sb", iota_in, [128, CPC * 128], fp16)

            rg = [list(range(NCORES))]
            dma_sems = [nc.alloc_semaphore(f"gsem{q}") for q in range(4)]

            # ---- g0 = (dinv*x) @ W1; half B first, AG as each half ends ----
            # AG(0,B) is emitted right away (layer 1 needs it first); AG(0,A)
            # is deferred into the L1-pass1 loop so its Pool-queue dispatch
            # (which waits on g0-half-A) doesn't block early gather issues.
            for half in (1, 0):
                for q in range(2):
                    t0q = q * (HT // 2)
                    t1q = HT if q == 1 else HT // 2
                    if t1q == t0q:
                        continue
                    QT = (HT + 1) // 2
                    xbs = []
                    for kk in range(4):
                        xb = xt_pool.tile([128, QT * 128], fp16, tag='xb',
                                          name=f'xb_{half}_{q}_{kk}')
                        nc.sync.dma_start(
                            out=xb[:, :(t1q - t0q) * 128],
                            in_=xT_in[kk * 128:(kk + 1) * 128,
                                      half * NH + t0q * 128:
                                      half * NH + t1q * 128])
                        xbs.append(xb)
                    for iq in range(t1q - t0q):
                        ii = t0q + iq
                        i = half * HT + ii
                        ps = psA.tile([128, 128], fp32, tag='agg',
                                      name=f'g0ps_{i}')
                        for kk in range(4):
                            nc.tensor.matmul(
                                ps[:], lhsT=xbs[kk][:, iq * 128:(iq + 1) * 128],
                                rhs=w1_sb[:, kk * 128:(kk + 1) * 128],
                                start=(kk == 0), stop=(kk == 3))
                        g0t = g0sb_pool.tile([128, 128], fp16)
                        nc.scalar.copy(out=g0t[:], in_=ps[:])
                        nc.sync.dma_start(
                            out=g_loc[(0, half)][ii * 128:(ii + 1) * 128, :],
                            in_=g0t[:])
                if half == 1:
                    nc.gpsimd.collective_compute(
                        "AllGather", mybir.AluOpType.bypass, replica_groups=rg,
                        ins=[g_loc[(0, half)].opt()],
                        outs=[g_half[(0, half)].opt()])

            # ---- two GCN layers; tiles half-B first ----
            TILE_ORDER = list(range(HT, TPC)) + list(range(HT))
            for layer in (1, 2):
                L = 0 if layer == 1 else 1
                in_ap = {}
                for h in range(4):
                    gv = g_half[(L, h // 2)][:].rearrange(
                        "(u two) d -> u (two d)", two=2)
                    in_ap[h] = gv[:, (h % 2) * 128:(h % 2) * 128 + 128]

                pos = {h: 0 for h in range(4)}
                tiles_cur = {h: None for h in range(4)}
                state = {'cc': 0}
                pool_ps = None
                if layer == 2:
                    pool_ps = psP.tile([64, 128], fp32, name='poolps')

                def next_chunk(h, layer=layer, in_ap=in_ap, pos=pos,
                               tiles_cur=tiles_cur, state=state):
                    p = pos[h]
                    if tiles_cur[h] is None or p % CPC == 0:
                        c0 = (p // CPC) * CPC
                        msg = msg_pool.tile([128, CPC * 128], fp16, tag='msg',
                                            name=f'msg_{layer}_{h}_{c0}')
                        qn = state['cc'] % 4
                        nc.gpsimd.dma_gather(
                            msg[:].rearrange("p (k c) -> p k c", c=128),
                            in_ap[h],
                            idx_sb[h][:, c0 * 8:(c0 + CPC) * 8],
                            CPC * 128, CPC * 128, 128,
                            elem_step=256, single_packet=True,
                            queue_num=qn)
                        state['cc'] += 1
                        sbt = s_pool.tile([128, CPC * 128], fp8, tag='s',
                                          name=f's_{layer}_{h}_{c0}')
                        nc.vector.tensor_tensor(
                            out=sbt[:].rearrange("p (k c) -> p k c", c=128),
                            in0=lid_sb[h][:, c0:c0 + CPC].to_broadcast(
                                [128, CPC, 128]),
                            in1=iota_sb[:].rearrange("p (k c) -> p k c", c=128),
                            op=iseq)
                        tiles_cur[h] = (msg, sbt, c0)
                    pos[h] = p + 1
                    msg, sbt, c0 = tiles_cur[h]
                    jj = p - c0
                    return (msg[:, jj * 128:(jj + 1) * 128],
                            sbt[:, jj * 128:(jj + 1) * 128])

                # ---- pass 1: half-B sources (streams 2,3) -> SBUF partial ----
                parts = {}
                defer_li = 20 if layer == 1 else 45
                for li, i in enumerate(TILE_ORDER):
                    if li == defer_li:
                        # deferred AG of the previous table's A half: by now
                        # its producer is long done, so the Pool dispatch
                        # doesn't stall the gather pipeline.
                        nc.gpsimd.collective_compute(
                            "AllGather", mybir.AluOpType.bypass,
                            replica_groups=rg,
                            ins=[g_loc[(L, 0)].opt()],
                            outs=[g_half[(L, 0)].opt()])
                    cs = [int(cslot[h][i]) for h in range(4)]
                    cB = cs[2] + cs[3]
                    agg_ps = psA.tile([128, 128], fp32, tag='agg',
                                      name=f'aggB_{layer}_{i}')
                    k = 0
                    for h in (2, 3):
                        for _ in range(cs[h]):
                            m_ap, s_ap = next_chunk(h)
                            if layer == 1:
                                nc.tensor.matmul(agg_ps[:], lhsT=s_ap,
                                                 rhs=m_ap, start=(k == 0),
                                                 stop=(k == cB - 1))
                            else:
                                nc.tensor.matmul(agg_ps[:], lhsT=m_ap,
                                                 rhs=s_ap, start=(k == 0),
                                                 stop=(k == cB - 1))
                            k += 1
                    part = part_pool.tile([128, 128], fp16, tag='part',
                                          name=f'part_{layer}_{i}')
                    nc.scalar.copy(out=part[:], in_=agg_ps[:])
                    parts[i] = part

                # ---- pass 2: half-A sources (streams 0,1) + epilogue ----
                for li, i in enumerate(TILE_ORDER):
                    cs = [int(cslot[h][i]) for h in range(4)]
                    cA = cs[0] + cs[1]
                    agg_ps = psA.tile([128, 128], fp32, tag='agg',
                                      name=f'agg_{layer}_{i}')
                    nc.tensor.matmul(agg_ps[:], lhsT=ident_sb[:],
                                     rhs=parts[i][:], start=True, stop=False)
                    k = 0
                    for h in (0, 1):
                        for _ in range(cs[h]):
                            m_ap, s_ap = next_chunk(h)
                            if layer == 1:
                                nc.tensor.matmul(agg_ps[:], lhsT=s_ap,
                                                 rhs=m_ap,
                                                 start=False, stop=False)
                            else:
                                nc.tensor.matmul(agg_ps[:], lhsT=m_ap,
                                                 rhs=s_ap, start=False,
                                                 stop=(k == cA - 1))
                            k += 1
                    # ---- tile epilogue ----
                    rd = rdinv_sb[0:1, i * 128:(i + 1) * 128]
                    if layer == 1:
                        nc.tensor.matmul(agg_ps[:], lhsT=rd, rhs=b1_sb[0:1, :],
                                         start=False, stop=True)
                        h1t = h1sb_pool.tile([128, 128], fp16)
                        nc.scalar.activation(
                            out=h1t[:], in_=agg_ps[:], func=Relu,
                            scale=dinv2_sb[:, i:i + 1])
                        half = 1 if i >= HT else 0
                        ii = i - half * HT
                        nc.sync.dma_start(
                            out=g_loc[(1, half)][ii * 128:(ii + 1) * 128, :],
                            in_=h1t[:])
                        if li == 70:
                            # AG of the next table's B half: emitted well
                            # after the last B-tile epilogue so the Pool
                            # dispatch doesn't block A-tile gather prefetch.
                            nc.gpsimd.collective_compute(
                                "AllGather", mybir.AluOpType.bypass,
                                replica_groups=rg,
                                ins=[g_loc[(1, 1)].opt()],
                                outs=[g_half[(1, 1)].opt()])
                    else:
                        aggt = aggt_pool.tile([128, 128], fp16)
                        nc.scalar.copy(out=aggt[:], in_=agg_ps[:])
                        h2ps = psB.tile([128, 128], fp32, tag='h2',
                                        name=f'h2ps_{i}')
                        nc.tensor.matmul(h2ps[:], lhsT=aggt[:], rhs=w2_sb[:],
                                         start=True, stop=False)
                        nc.tensor.matmul(h2ps[:], lhsT=rd, rhs=b2_sb[0:1, :],
                                         start=False, stop=True)
                        h2t = h2sb_pool.tile([128, 128], fp16)
                        nc.scalar.activation(
                            out=h2t[:], in_=h2ps[:], func=Relu,
                            scale=dinv1_sb[:, i:i + 1])
                        nc.tensor.matmul(pool_ps[:],
                                         lhsT=pmat_sb[:, i * NG:(i + 1) * NG],
                                         rhs=h2t[:],
                                         start=(li == 0), stop=(li == TPC - 1))

            pooled_t = osb_pool.tile([64, 128], fp32)
            nc.scalar.copy(out=pooled_t[:], in_=pool_ps[:])
            nc.sync.dma_start(out=out_dram, in_=pooled_t[:])

    nc.compile()
    return nc


def _make_in_maps(inputs, gp):
    x = np.asarray(inputs['x'])
    W1 = np.asarray(inputs['W1'])
    b1 = np.asarray(inputs['b1'])
    W2 = np.asarray(inputs['W2'])
    b2 = np.asarray(inputs['b2'])
    dinv = gp['dinv']

    xs = np.zeros((NP, D), np.float16)
    xs[:N] = (x.astype(np.float64) * dinv[:, None]).astype(np.float16)
    w1r = np.ascontiguousarray(
        W1.astype(np.float16).reshape(4, 128, 128).transpose(1, 0, 2)
    ).reshape(128, 4 * 128)
    w2r = W2.astype(np.float16)
    b1r = b1.astype(np.float16).reshape(1, 128)
    b2r = b2.astype(np.float16).reshape(1, 128)
    iota = np.tile(np.arange(128, dtype=np.float16)[None, :], (128, CPC))

    in_maps = []
    for c in range(NCORES):
        no = gp['node_order'][c]
        xT = np.ascontiguousarray(xs[no].T)
        im = {
            "xT": xT,
            "ident": np.eye(128, dtype=np.float16),
            "w1": w1r, "w2": w2r, "b1": b1r, "b2": b2r,
            "rdinv": gp['rdinv'][no].reshape(1, NPC),
            "dinv2": gp['dinv2'][no].reshape(TPC, 128).T.copy(),
            "dinv1": gp['dinv1'][no].reshape(TPC, 128).T.copy(),
            "pmat": np.ascontiguousarray(
                gp['pmat'][no].reshape(TPC, 128, NG).transpose(1, 0, 2)
            ).reshape(128, TPC * NG),
            "iota": iota,
        }
        for h in range(4):
            im[f"idx{h}"] = gp['idx'][h][c]
            im[f"lid{h}"] = gp['lid'][h][c]
        in_maps.append(im)
    return in_maps


def _get_built(inputs):
    ei = np.asarray(inputs['edge_index'])
    key = hash((ei.shape, ei[0, :50].tobytes(), ei[1, -50:].tobytes()))
    if _cache.get('key') != key:
        gp = _host_prep_graph(inputs['edge_index'], inputs['batch'])
        nc = _build_nc(gp)
        _cache.update(key=key, gp=gp, nc=nc)
    return _cache['nc'], _cache['gp']


def kernel(run_kwargs=None, **inputs):
    from concourse.bass_utils import run_bass_kernel_spmd
    nc, gp = _get_built(inputs)
    in_maps = _make_in_maps(inputs, gp)
    res = run_bass_kernel_spmd(nc, in_maps, list(range(NCORES)),
                               **(run_kwargs or {}))
    out = np.zeros((64, 128), np.float64)
    for r in res.results:
        out += r["pooled"].astype(np.float64)
    if run_kwargs:
        _cache['last_res'] = res
    return out.astype(np.float32)



# revision 24
# speedup vs baseline: 1.0057x; 1.0057x over previous
"""GCN (2-layer) + global mean pool on 8 Trainium2 NeuronCores.

Strategy
--------
Nodes are padded to 100352 = 784 tiles of 128; dest tiles are split
contiguously across 8 cores (98 each).  Per layer the replicated fp16 table

    g[n] = dinv[n] * h[n]        ([100352, 128], AllGather per layer)

is gathered per edge with the GPSIMD `dma_gather` custom DMA (int16 indices,
4 SWDGE queues round-robin).  int16 addressing uses 512-byte pairs
(idx = half-local node // 2); edges are split per dest tile into
(source-table-half x source-parity) streams so every call is parity-pure.
Calls are 1024 indices (CPC=8 chunks) with single_packet=True: 64
descriptors per SDMA engine, the packet ceiling.  Padding slots repeat the
preceding real index (open-row HBM hit) instead of fetching row 0.

Edges sorted by destination are processed in 128-edge chunks: the 0/1
selection matrix S[e, c] = (lid[e] == c) is built on the vector engine in
fp8; aggregation is a PE matmul accumulated per dest-tile in PSUM.
deg^-1/2 is folded into the table (source side) and activation scales /
K=1 bias matmuls (dest side), so no per-edge multiply exists anywhere.

Each layer runs TWO PASSES over its dest tiles: pass 1 aggregates only
half-B-source streams (needs only the half-B AllGather, which lands first)
into fp16 SBUF partials; pass 2 re-injects the partial with an identity
matmul, adds the half-A streams, and runs the epilogue.  This removes the
table AllGathers from the gather critical path.  AllGather *emission* is
deferred into the consuming loops so its Pool-queue dispatch (in-order!)
never blocks gather issue.

Layer 1 stores relu(dinv^2*agg + dinv*b1) as the next table; layer 2
aggregates transposed (lhsT=msg, rhs=S) so W2 consumes the aggregate as
stationary operand; mean pooling is one more indicator matmul accumulated
over all tiles in PSUM; the host sums the 8 per-core [64,128] partials.

Measured: ~1.41-1.44ms HW exec (baseline 1.86ms); gather DMA active ~1.0ms
(129.5MB of random 256B HBM reads at ~130GB/s/core) is the roofline wall.
"""

import sys

if '/opt/trn_rl_repo' not in sys.path:
    sys.path.insert(0, '/opt/trn_rl_repo')

import numpy as np

NCORES = 8
N = 100000
NP = 100352          # 784 tiles of 128
NTILES = 784
TPC = NTILES // NCORES   # 98 dest tiles per core
NPC = TPC * 128          # 12544 nodes per core
D = 512
H = 128
NG = 64
CPC = 8              # chunks per gather call / S-build batch (1024 idxs;
                     # 64 descs/engine = single-packet ceiling)
NSTR = 2             # gather streams (source parity)

_cache = {}


def _wrap_idx(vals16, tc_pad):
    """[128, tc_pad] int16 per-(lane, chunk) values -> dma_gather wrapped
    layout [128, tc_pad*8]: within-call index i = (q%16)*128 + p lives at
    [p%16 (+16r), q*8 + p//16]."""
    lanes = np.arange(128)
    out16 = np.zeros((16, tc_pad * 8), np.int16)
    cols = (lanes[:, None] // 16) + np.arange(tc_pad)[None, :] * 8
    rows = (lanes % 16)[:, None].repeat(tc_pad, axis=1)
    out16[rows, cols] = vals16
    return np.tile(out16, (8, 1))


def _host_prep_graph(edge_index, batch):
    ei = np.asarray(edge_index)
    row = np.concatenate([ei[0], np.arange(N, dtype=np.int64)])
    col = np.concatenate([ei[1], np.arange(N, dtype=np.int64)])
    deg = np.bincount(col, minlength=N).astype(np.float64)
    dinv = 1.0 / np.sqrt(deg)

    # ---- balanced dest-tile assignment (LPT), slot-paired by size ----
    tile_tot = np.bincount(col >> 7, minlength=NTILES)
    order_t = np.argsort(-tile_tot, kind='stable')
    loads = np.zeros(NCORES, np.int64)
    counts = np.zeros(NCORES, np.int64)
    assign = [[] for _ in range(NCORES)]
    for t in order_t:
        cand = np.where(counts < TPC)[0]
        cbest = cand[np.argmin(loads[cand])]
        assign[cbest].append(int(t))
        loads[cbest] += tile_tot[t]
        counts[cbest] += 1
    # big tiles first within each core -> slot i pairs similar sizes
    slot_tile = np.array([sorted(a, key=lambda t: -tile_tot[t])
                          for a in assign])          # [NCORES, TPC]

    # ---- node -> table-row remap (half-major, slot order per core) ----
    HT = TPC // 2
    NH = HT * 128
    HNP = NP // 2
    remap = np.zeros(NP, np.int64)
    for c in range(NCORES):
        for i in range(TPC):
            t = slot_tile[c, i]
            half = 1 if i >= HT else 0
            base = half * HNP + c * NH + (i - half * HT) * 128
            remap[t * 128:(t + 1) * 128] = base + np.arange(128)

    # ---- edge streams: (source table half)*2 + source parity ----
    rr = remap[row]
    es = (rr >= HNP) * 2 + (row & 1)
    tile_of_e = col >> 7
    key = tile_of_e * 4 + es
    order = np.argsort(key, kind='stable')
    row_s = row[order].astype(np.int64)
    col_s = col[order].astype(np.int64)
    es_s = es[order]

    tp = (col_s >> 7) * 4 + es_s
    tp_cnt = np.bincount(tp, minlength=NTILES * 4).reshape(NTILES, 4)
    tp_start = np.zeros(NTILES * 4 + 1, np.int64)
    np.cumsum(tp_cnt.reshape(-1), out=tp_start[1:])
    tp_start = tp_start[:-1].reshape(NTILES, 4)

    def pad16(x):
        return ((x + CPC - 1) // CPC) * CPC

    # chunks are laid out in the device's processing order: half-B slots
    # first, then half-A (matches TILE_ORDER in _build_nc)
    proc_order = list(range(HT, TPC)) + list(range(HT))
    cslot, tc_pad, bases = [], [], []
    for h in range(4):
        cnt_h = tp_cnt[slot_tile, h]                 # [NCORES, TPC]
        cs = np.ceil(cnt_h / 128).astype(np.int64).max(axis=0)
        cslot.append(cs)
        tc_pad.append(pad16(max(int(cs.sum()), 1)))
        b = np.zeros(TPC, np.int64)
        acc = 0
        for i in proc_order:
            b[i] = acc
            acc += cs[i]
        bases.append(b)

    idxs = [np.zeros((NCORES, 128, tc_pad[h]), np.int16) for h in range(4)]
    lids = [np.full((NCORES, 128, tc_pad[h]), -1.0, np.float16)
            for h in range(4)]
    for c in range(NCORES):
        for i in range(TPC):
            t = slot_tile[c, i]
            for h in range(4):
                s, n = tp_start[t, h], tp_cnt[t, h]
                if n == 0:
                    continue
                src = row_s[s:s + n]
                loc = (col_s[s:s + n] - (t << 7)).astype(np.float16)
                j0 = bases[h][i]
                ch = np.arange(n) // 128 + j0
                ln = np.arange(n) % 128
                idxs[h][c, ln, ch] = (remap[src] % HNP) >> 1
                lids[h][c, ln, ch] = loc

    # Padding slots (lid == -1) keep idx 0 by construction, which makes every
    # padding descriptor hammer table row 0 (one HBM bank).  Forward-fill them
    # with the preceding real index in unwrapped (chunk, lane) order so the
    # padding fetch hits the row the engine just read (open-row hit).
    for h in range(4):
        for c in range(NCORES):
            flat_i = idxs[h][c].T.reshape(-1).copy()      # (ch, ln) order
            flat_m = (lids[h][c].T.reshape(-1) == -1.0)
            last = np.where(~flat_m, np.arange(flat_i.size), -1)
            np.maximum.accumulate(last, out=last)
            src = np.where(last >= 0, flat_i[np.maximum(last, 0)], flat_i)
            flat_i[flat_m] = src[flat_m]
            idxs[h][c] = flat_i.reshape(tc_pad[h], 128).T

    idx_w = [np.stack([_wrap_idx(idxs[h][c], tc_pad[h])
                       for c in range(NCORES)]) for h in range(4)]

    # ---- per-node dest-side arrays, in slot order per core ----
    dinv_p = np.ones(NP, np.float64)
    dinv_p[:N] = dinv
    rdinv_n = np.zeros(NP, np.float16)
    rdinv_n[:N] = (1.0 / dinv).astype(np.float16)
    dinv2_n = (dinv_p ** 2).astype(np.float32)
    dinv1_n = dinv_p.astype(np.float32)
    b = np.asarray(batch).astype(np.int64)
    cnt_g = np.bincount(b, minlength=NG).astype(np.float64)
    invcnt = 1.0 / np.maximum(cnt_g, 1.0)
    pmat_n = np.zeros((NP, NG), np.float16)
    pmat_n[np.arange(N), b] = invcnt[b].astype(np.float16)

    # node order per core = concat of its slot tiles
    node_order = np.zeros((NCORES, NPC), np.int64)
    for c in range(NCORES):
        node_order[c] = (slot_tile[c][:, None] * 128 +
                         np.arange(128)[None, :]).reshape(-1)

    return dict(
        dinv=dinv, cslot=cslot, tc_pad=tc_pad, idx=idx_w, lid=lids,
        node_order=node_order,
        rdinv=rdinv_n, dinv2=dinv2_n, dinv1=dinv1_n, pmat=pmat_n,
    )


def _build_nc(gp):
    import concourse.bass as bass
    import concourse.bacc as bacc
    import concourse.mybir as mybir
    import concourse.tile as tile

    fp16 = mybir.dt.float16
    fp8 = mybir.dt.float8e4
    fp32 = mybir.dt.float32
    i16 = mybir.dt.int16
    Relu = mybir.ActivationFunctionType.Relu
    iseq = mybir.AluOpType.is_equal

    tc_pad = gp['tc_pad']
    cslot = gp['cslot']
    STREAM_ORDER = (2, 3, 0, 1)      # half-B streams first (its AG lands first)

    nc = bacc.Bacc("TRN2", target_bir_lowering=False, debug=False,
                   num_devices=NCORES, num_swdge_queues=4)

    xT_in = nc.dram_tensor("xT", [D, NPC], fp16, kind="ExternalInput").ap()
    idx_in = [nc.dram_tensor(f"idx{h}", [128, tc_pad[h] * 8], i16,
                             kind="ExternalInput").ap() for h in range(4)]
    lid_in = [nc.dram_tensor(f"lid{h}", [128, tc_pad[h]], fp16,
                             kind="ExternalInput").ap() for h in range(4)]
    ident_in = nc.dram_tensor("ident", [128, 128], fp16,
                              kind="ExternalInput").ap()
    w1_in = nc.dram_tensor("w1", [128, 4 * 128], fp16, kind="ExternalInput").ap()
    w2_in = nc.dram_tensor("w2", [128, 128], fp16, kind="ExternalInput").ap()
    b1_in = nc.dram_tensor("b1", [1, 128], fp16, kind="ExternalInput").ap()
    b2_in = nc.dram_tensor("b2", [1, 128], fp16, kind="ExternalInput").ap()
    rdinv_in = nc.dram_tensor("rdinv", [1, NPC], fp16, kind="ExternalInput").ap()
    dinv2_in = nc.dram_tensor("dinv2", [128, TPC], fp32, kind="ExternalInput").ap()
    dinv1_in = nc.dram_tensor("dinv1", [128, TPC], fp32, kind="ExternalInput").ap()
    pmat_in = nc.dram_tensor("pmat", [128, TPC * NG], fp16,
                             kind="ExternalInput").ap()
    iota_in = nc.dram_tensor("iota", [128, CPC * 128], fp16,
                             kind="ExternalInput").ap()
    out_dram = nc.dram_tensor("pooled", [64, 128], fp32,
                              kind="ExternalOutput").ap()

    with tile.TileContext(nc) as tcx:
        import contextlib
        ctx = contextlib.ExitStack()
        with ctx:
            dram = ctx.enter_context(tcx.tile_pool(name="dram", bufs=1, space="DRAM"))
            cpool = ctx.enter_context(tcx.tile_pool(name="const", bufs=1))
            xt_pool = ctx.enter_context(tcx.tile_pool(name="xt", bufs=4))
            g0sb_pool = ctx.enter_context(tcx.tile_pool(name="g0sb", bufs=3))
            msg_pool = ctx.enter_context(tcx.tile_pool(name="msg", bufs=26))
            s_pool = ctx.enter_context(tcx.tile_pool(name="spool", bufs=26))
            h1sb_pool = ctx.enter_context(tcx.tile_pool(name="h1sb", bufs=3))
            part_pool = ctx.enter_context(tcx.tile_pool(name="part", bufs=TPC))
            aggt_pool = ctx.enter_context(tcx.tile_pool(name="aggt", bufs=3))
            h2sb_pool = ctx.enter_context(tcx.tile_pool(name="h2sb", bufs=3))
            osb_pool = ctx.enter_context(tcx.tile_pool(name="osb", bufs=1))
            psA = ctx.enter_context(tcx.tile_pool(name="psA", bufs=5, space="PSUM"))
            psB = ctx.enter_context(tcx.tile_pool(name="psB", bufs=2, space="PSUM"))
            psP = ctx.enter_context(tcx.tile_pool(name="psP", bufs=1, space="PSUM"))

            HT = TPC // 2
            NH = HT * 128
            HNP = NP // 2
            g_loc = {}
            g_half = {}
            for L in (0, 1):
                for hf in (0, 1):
                    g_loc[(L, hf)] = dram.tile([NH, 128], fp16,
                                               name=f"g{L}loc{hf}")
                    g_half[(L, hf)] = dram.tile([HNP, 128], fp16,
                                                addr_space="Shared",
                                                name=f"g{L}half{hf}")

            def cload(name, ap_in, shape, dt):
                t = cpool.tile(shape, dt, name=name)
                nc.sync.dma_start(out=t[:], in_=ap_in)
                return t

            ident_sb = cload("ident_sb", ident_in, [128, 128], fp16)
            idx_sb = [cload(f"idx_sb{h}", idx_in[h], [128, tc_pad[h] * 8], i16)
                      for h in range(4)]
            lid_sb = [cload(f"lid_sb{h}", lid_in[h], [128, tc_pad[h]], fp16)
                      for h in range(4)]
            w1_sb = cload("w1_sb", w1_in, [128, 4 * 128], fp16)
            w2_sb = cload("w2_sb", w2_in, [128, 128], fp16)
            b1_sb = cload("b1_sb", b1_in, [1, 128], fp16)
            b2_sb = cload("b2_sb", b2_in, [1, 128], fp16)
            rdinv_sb = cload("rdinv_sb", rdinv_in, [1, NPC], fp16)
            dinv2_sb = cload("dinv2_sb", dinv2_in, [128, TPC], fp32)
            dinv1_sb = cload("dinv1_sb", dinv1_in, [128, TPC], fp32)
            pmat_sb = cload("pmat_sb", pmat_in, [128, TPC * NG], fp16)
            iota_sb = cload("iota_sb", iota_in, [128, CPC * 128], fp16)

            rg = [list(range(NCORES))]
            dma_sems = [nc.alloc_semaphore(f"gsem{q}") for q in range(4)]

            # ---- g0 = (dinv*x) @ W1; half B first, AG as each half ends ----
            # AG(0,B) is emitted right away (layer 1 needs it first); AG(0,A)
            # is deferred into the L1-pass1 loop so its Pool-queue dispatch
            # (which waits on g0-half-A) doesn't block early gather issues.
            for half in (1, 0):
                for q in range(2):
                    t0q = q * (HT // 2)
                    t1q = HT if q == 1 else HT // 2
                    if t1q == t0q:
                        continue
                    QT = (HT + 1) // 2
                    xbs = []
                    for kk in range(4):
                        xb = xt_pool.tile([128, QT * 128], fp16, tag='xb',
                                          name=f'xb_{half}_{q}_{kk}')
                        nc.sync.dma_start(
                            out=xb[:, :(t1q - t0q) * 128],
                            in_=xT_in[kk * 128:(kk + 1) * 128,
                                      half * NH + t0q * 128:
                                      half * NH + t1q * 128])
                        xbs.append(xb)
                    for iq in range(t1q - t0q):
                        ii = t0q + iq
                        i = half * HT + ii
                        ps = psA.tile([128, 128], fp32, tag='agg',
                                      name=f'g0ps_{i}')
                        for kk in range(4):
                            nc.tensor.matmul(
                                ps[:], lhsT=xbs[kk][:, iq * 128:(iq + 1) * 128],
                                rhs=w1_sb[:, kk * 128:(kk + 1) * 128],
                                start=(kk == 0), stop=(kk == 3))
                        g0t = g0sb_pool.tile([128, 128], fp16)
                        nc.scalar.copy(out=g0t[:], in_=ps[:])
                        nc.sync.dma_start(
                            out=g_loc[(0, half)][ii * 128:(ii + 1) * 128, :],
                            in_=g0t[:])
                if half == 1:
                    nc.gpsimd.collective_compute(
                        "AllGather", mybir.AluOpType.bypass, replica_groups=rg,
                        ins=[g_loc[(0, half)].opt()],
                        outs=[g_half[(0, half)].opt()])

            # ---- two GCN layers; tiles half-B first ----
            TILE_ORDER = list(range(HT, TPC)) + list(range(HT))
            for layer in (1, 2):
                L = 0 if layer == 1 else 1
                in_ap = {}
                for h in range(4):
                    gv = g_half[(L, h // 2)][:].rearrange(
                        "(u two) d -> u (two d)", two=2)
                    in_ap[h] = gv[:, (h % 2) * 128:(h % 2) * 128 + 128]

                pos = {h: 0 for h in range(4)}
                tiles_cur = {h: None for h in range(4)}
                state = {'cc': 0}
                pool_ps = None
                if layer == 2:
                    pool_ps = psP.tile([64, 128], fp32, name='poolps')

                def next_chunk(h, layer=layer, in_ap=in_ap, pos=pos,
                               tiles_cur=tiles_cur, state=state):
                    p = pos[h]
                    if tiles_cur[h] is None or p % CPC == 0:
                        c0 = (p // CPC) * CPC
                        msg = msg_pool.tile([128, CPC * 128], fp16, tag='msg',
                                            name=f'msg_{layer}_{h}_{c0}')
                        qn = state['cc'] % 4
                        nc.gpsimd.dma_gather(
                            msg[:].rearrange("p (k c) -> p k c", c=128),
                            in_ap[h],
                            idx_sb[h][:, c0 * 8:(c0 + CPC) * 8],
                            CPC * 128, CPC * 128, 128,
                            elem_step=256, single_packet=True,
                            queue_num=qn)
                        state['cc'] += 1
                        sbt = s_pool.tile([128, CPC * 128], fp8, tag='s',
                                          name=f's_{layer}_{h}_{c0}')
                        nc.vector.tensor_tensor(
                            out=sbt[:].rearrange("p (k c) -> p k c", c=128),
                            in0=lid_sb[h][:, c0:c0 + CPC].to_broadcast(
                                [128, CPC, 128]),
                            in1=iota_sb[:].rearrange("p (k c) -> p k c", c=128),
                            op=iseq)
                        tiles_cur[h] = (msg, sbt, c0)
                    pos[h] = p + 1
                    msg, sbt, c0 = tiles_cur[h]
                    jj = p - c0
                    return (msg[:, jj * 128:(jj + 1) * 128],
                            sbt[:, jj * 128:(jj + 1) * 128])

                # ---- pass 1: half-B sources (streams 2,3) -> SBUF partial ----
                parts = {}
                defer_li = 20 if layer == 1 else 45
                for li, i in enumerate(TILE_ORDER):
                    if li == defer_li:
                        # deferred AG of the previous table's A half: by now
                        # its producer is long done, so the Pool dispatch
                        # doesn't stall the gather pipeline.
                        nc.gpsimd.collective_compute(
                            "AllGather", mybir.AluOpType.bypass,
                            replica_groups=rg,
                            ins=[g_loc[(L, 0)].opt()],
                            outs=[g_half[(L, 0)].opt()])
                    cs = [int(cslot[h][i]) for h in range(4)]
                    cB = cs[2] + cs[3]
                    agg_ps = psA.tile([128, 128], fp32, tag='agg',
                                      name=f'aggB_{layer}_{i}')
                    k = 0
                    for h in (2, 3):
                        for _ in range(cs[h]):
                            m_ap, s_ap = next_chunk(h)
                            if layer == 1:
                                nc.tensor.matmul(agg_ps[:], lhsT=s_ap,
                                                 rhs=m_ap, start=(k == 0),
                                                 stop=(k == cB - 1))
                            else:
                                nc.tensor.matmul(agg_ps[:], lhsT=m_ap,
                                                 rhs=s_ap, start=(k == 0),
                                                 stop=(k == cB - 1))
                            k += 1
                    part = part_pool.tile([128, 128], fp16, tag='part',
                                          name=f'part_{layer}_{i}')
                    nc.scalar.copy(out=part[:], in_=agg_ps[:])
                    parts[i] = part

                # ---- pass 2: half-A sources (streams 0,1) + epilogue ----
                for li, i in enumerate(TILE_ORDER):
                    cs = [int(cslot[h][i]) for h in range(4)]
                    cA = cs[0] + cs[1]
                    agg_ps = psA.tile([128, 128], fp32, tag='agg',
                                      name=f'agg_{layer}_{i}')
                    nc.tensor.matmul(agg_ps[:], lhsT=ident_sb[:],
                                     rhs=parts[i][:], start=True, stop=False)
                    k = 0
                    for h in (0, 1):
                        for _ in range(cs[h]):
                            m_ap, s_ap = next_chunk(h)
                            if layer == 1:
                                nc.tensor.matmul(agg_ps[:], lhsT=s_ap,
                                                 rhs=m_ap,
                                                 start=False, stop=False)
                            else:
                                nc.tensor.matmul(agg_ps[:], lhsT=m_ap,
                                                 rhs=s_ap, start=False,
                                                 stop=(k == cA - 1))
                            k += 1
                    # ---- tile epilogue ----
                    rd = rdinv_sb[0:1, i * 128:(i + 1) * 128]
                    if layer == 1:
                        nc.tensor.matmul(agg_ps[:], lhsT=rd, rhs=b1_sb[0:1, :],
                                         start=False, stop=True)
                        h1t = h1sb_pool.tile([128, 128], fp16)
                        nc.scalar.activation(
                            out=h1t[:], in_=agg_ps[:], func=Relu,
                            scale=dinv2_sb[:, i:i + 1])
                        half = 1 if i >= HT else 0
                        ii = i - half * HT
                        nc.sync.dma_start(
                            out=g_loc[(1, half)][ii * 128:(ii + 1) * 128, :],
                            in_=h1t[:])
                        if li == 70:
                            # AG of the next table's B half: emitted well
                            # after the last B-tile epilogue so the Pool
                            # dispatch doesn't block A-tile gather prefetch.
                            nc.gpsimd.collective_compute(
                                "AllGather", mybir.AluOpType.bypass,
                                replica_groups=rg,
                                ins=[g_loc[(1, 1)].opt()],
                                outs=[g_half[(1, 1)].opt()])
                    else:
                        aggt = aggt_pool.tile([128, 128], fp16)
                        nc.scalar.copy(out=aggt[:], in_=agg_ps[:])
                        h2ps = psB.tile([128, 128], fp32, tag='h2',
                                        name=f'h2ps_{i}')
                        nc.tensor.matmul(h2ps[:], lhsT=aggt[:], rhs=w2_sb[:],
                                         start=True, stop=False)
                        nc.tensor.matmul(h2ps[:], lhsT=rd, rhs=b2_sb[0:1, :],
                                         start=False, stop=True)
                        h2t = h2sb_pool.tile([128, 128], fp16)
                        nc.scalar.activation(
                            out=h2t[:], in_=h2ps[:], func=Relu,
                            scale=dinv1_sb[:, i:i + 1])
                        nc.tensor.matmul(pool_ps[:],
                                         lhsT=pmat_sb[:, i * NG:(i + 1) * NG],
                                         rhs=h2t[:],
                                         start=(li == 0), stop=(li == TPC - 1))

            pooled_t = osb_pool.tile([64, 128], fp32)
            nc.scalar.copy(out=pooled_t[:], in_=pool_ps[:])
            nc.sync.dma_start(out=out_dram, in_=pooled_t[:])

    nc.compile()
    return nc


def _make_in_maps(inputs, gp):
    x = np.asarray(inputs['x'])
    W1 = np.asarray(inputs['W1'])
    b1 = np.asarray(inputs['b1'])
    W2 = np.asarray(inputs['W2'])
    b2 = np.asarray(inputs['b2'])
    dinv = gp['dinv']

    xs = np.zeros((NP, D), np.float16)
    xs[:N] = (x.astype(np.float64) * dinv[:, None]).astype(np.float16)
    w1r = np.ascontiguousarray(
        W1.astype(np.float16).reshape(4, 128, 128).transpose(1, 0, 2)
    ).reshape(128, 4 * 128)
    w2r = W2.astype(np.float16)
    b1r = b1.astype(np.float16).reshape(1, 128)
    b2r = b2.astype(np.float16).reshape(1, 128)
    iota = np.tile(np.arange(128, dtype=np.float16)[None, :], (128, CPC))

    in_maps = []
    for c in range(NCORES):
        no = gp['node_order'][c]
        xT = np.ascontiguousarray(xs[no].T)
        im = {
            "xT": xT,
            "ident": np.eye(128, dtype=np.float16),
            "w1": w1r, "w2": w2r, "b1": b1r, "b2": b2r,
            "rdinv": gp['rdinv'][no].reshape(1, NPC),
            "dinv2": gp['dinv2'][no].reshape(TPC, 128).T.copy(),
            "dinv1": gp['dinv1'][no].reshape(TPC, 128).T.copy(),
            "pmat": np.ascontiguousarray(
                gp['pmat'][no].reshape(TPC, 128, NG).transpose(1, 0, 2)
            ).reshape(128, TPC * NG),
            "iota": iota,
        }
        for h in range(4):
            im[f"idx{h}"] = gp['idx'][h][c]
            im[f"lid{h}"] = gp['lid'][h][c]
        in_maps.append(im)
    return in_maps


def _get_built(inputs):
    ei = np.asarray(inputs['edge_index'])
    key = hash((ei.shape, ei[0, :50].tobytes(), ei[1, -50:].tobytes()))
    if _cache.get('key') != key:
        gp = _host_prep_graph(inputs['edge_index'], inputs['batch'])
        nc = _build_nc(gp)
        _cache.update(key=key, gp=gp, nc=nc)
    return _cache['nc'], _cache['gp']


def kernel(run_kwargs=None, **inputs):
    from concourse.bass_utils import run_bass_kernel_spmd
    nc, gp = _get_built(inputs)
    in_maps = _make_in_maps(inputs, gp)
    res = run_bass_kernel_spmd(nc, in_maps, list(range(NCORES)),
                               **(run_kwargs or {}))
    out = np.zeros((64, 128), np.float64)
    for r in res.results:
        out += r["pooled"].astype(np.float64)
    if run_kwargs:
        _cache['last_res'] = res
    return out.astype(np.float32)

